# revision 64
# baseline (speedup 1.0000x reference)
"""CGCNN regressor on 8 trn2 NeuronCores.

Sharding: graphs 32/core -> contiguous node blocks; edges live on dst's core.
Per core, nodes are permuted into 52 "ranges" of 128 (degree-balanced bin
packing, <=512 edges/range); each range owns 4 edge chunks of 128 slots.
Per layer: h (fp16, x1/16) is AllGathered to a replicated pair-table
[26624, 256]; h[src] is fetched with one dma_gather(transpose=True) per
block; messages are fp16 matmuls (src + edge_attr + hi/lo dst-projection
expansion via host-precomputed one-hots) accumulating in PSUM. The dst
projection keeps ~fp32 precision via two fp16 planes of h and of Wdst.
sigmoid/softplus run block-batched (softplus = max(s, ln(1+exp(min(s,10))))
from the natural_log_exp table set). Aggregation is a one-hot fp16 matmul;
BN/silu/residual updates are quarter-batched. Pool/head run on 32
graphs/core; host concatenates the 8x[32] outputs.

Latency pipelining: the h table uses a slice-major pair numbering (range
slices [0:16)/[16:32)/[32:48)/[48:52) per core), so the next layer's
cast+stage+AllGather is emitted per-slice inside THIS layer's block loop
as soon as the slice's residual updates land; only the tiny 4-range tail
AllGather is exposed at the layer boundary. Block gathers are issued as
SWDGE prepare_only descs (desc-gen runs ahead on gpsimd) and fired with
trigger_dma, so the gather DMA overlaps compute instead of holding gpsimd.
"""

import os
import sys

import numpy as np

try:
    import concourse.bass as bass
except ImportError:  # grading env fallback
    sys.path.insert(0, "/opt/trn_rl_repo")
    import concourse.bass as bass

import concourse.mybir as mybir
import concourse.tile as tile
from concourse import bacc
from concourse.bass_utils import run_bass_kernel_spmd

F32 = np.float32
F16 = np.float16

# problem constants
N, E, H, ED, NG, NEMB, L = 50000, 200000, 128, 50, 256, 100, 6
C = 8               # cores
GPC = NG // C       # graphs per core
NT = 52             # node tiles (ranges) per core
N_LOC = NT * 128    # padded local nodes (6656)
CPR = 4             # chunks per range
NCHUNK = NT * CPR   # 208
NSLOT = NCHUNK * 128  # 26624 edge slots
CPB = 13            # chunks per gather block
NBLK = NCHUNK // CPB  # 8
SLOT_B = CPB * 128  # 1664 slots per block
RPQ = 13            # ranges per quarter
PAIRS = C * N_LOC // 2  # 26624 pair rows in the replicated h table
PAD_DST = 255       # dst sentinel for dummy slots (matches no one-hot row)
HSC = 1.0 / 16.0    # fp16 h-table scale (h stored as h*HSC; 16x in Wsrc)
WSC = 32.0          # wdst hi/lo plane scale (undone in the p evacuation)

_L_RUN = int(os.environ.get("KERNEL_LAYERS", str(L)))
_PREP = os.environ.get("KERNEL_PREP", "0") == "1"
_PLO = os.environ.get("KERNEL_PLO", "1") == "1"


# ---------------------------------------------------------------------------
# host-side preprocessing
# ---------------------------------------------------------------------------

def _wrap16(idx, pad_to):
    """int16 index tensor in dma_gather layout: [128, pad_to//16],
    slot i -> row i%16, col i//16; replicated 8x down the partitions."""
    a = np.full(pad_to, 0, np.int16)
    a[: len(idx)] = idx.astype(np.int16)
    w = a.reshape(pad_to // 16, 16).T  # [16, pad/16]
    return np.tile(w, (8, 1)).copy()


def _bn_fold(p, bias=None):
    gamma, beta, mean, var = [np.asarray(x, np.float64) for x in p]
    scale = gamma / np.sqrt(var + 1e-5)
    shift = beta - mean * scale
    if bias is not None:
        shift = shift + np.asarray(bias, np.float64) * scale
    return scale.astype(F32), shift.astype(F32)


def _rep(row, parts=128):
    row = np.asarray(row, F32).reshape(1, -1)
    return np.repeat(row, parts, axis=0).copy()


def _hilo(w, scale=1.0):
    w = np.asarray(w, np.float64) * scale
    hi = w.astype(F16)
    lo = (w - hi.astype(np.float64)).astype(F16)
    return hi, lo


def _prep(inputs):
    x_atom = np.asarray(inputs["x_atom"]).astype(np.int64)
    ei = np.asarray(inputs["edge_index"]).astype(np.int64)
    ea = np.asarray(inputs["edge_attr"]).astype(F32)
    batch = np.asarray(inputs["batch"]).astype(np.int64)
    src, dst = ei[0], ei[1]

    node_start = np.searchsorted(batch, np.arange(0, NG + 1, GPC))
    deg = np.bincount(dst, minlength=N)

    # global node -> (core, local id); degree-balanced FFD into NT ranges/core
    lid = np.empty(N, np.int64)
    core_of = np.empty(N, np.int64)
    for c in range(C):
        s, e = node_start[c], node_start[c + 1]
        nodes = np.arange(s, e)
        assert len(nodes) <= N_LOC, f"core {c}: {len(nodes)} > {N_LOC}"
        order = nodes[np.argsort(-deg[nodes], kind="stable")]
        cap_n = np.full(NT, 128, np.int64)
        cap_e = np.full(NT, CPR * 128, np.int64)
        pos = np.zeros(NT, np.int64)
        for g in order:
            d = deg[g]
            cand = np.where((cap_n > 0) & (cap_e >= d))[0]
            assert len(cand), f"core {c}: range packing failed (deg {d})"
            r = cand[np.argmax(cap_e[cand])]
            lid[g] = r * 128 + pos[r]
            pos[r] += 1
            cap_n[r] -= 1
            cap_e[r] -= d
        core_of[s:e] = c

    # slice-major pair address in the replicated table: range slices of
    # 16/16/16/4 ranges AllGather independently, so slice s of core c lands
    # at rows [8192*s + c*rows_s, ...) of hfull (rows_s = 1024 or 256).
    lid_r, lid_p = lid // 128, lid % 128
    sl_of = np.minimum(lid_r // 16, 3)
    rows_s = np.where(sl_of < 3, 1024, 256)
    gpair = sl_of * 8192 + core_of * rows_s + (lid_r - 16 * sl_of) * 64 \
        + lid_p // 2
    gpar = lid_p & 1

    in_maps = []
    for c in range(C):
        s, e = node_start[c], node_start[c + 1]
        slot_pair = np.zeros(NSLOT, np.int64)
        slot_par = np.zeros(NSLOT, F32)
        slot_dst = np.full(NSLOT, PAD_DST, np.int64)
        slot_ea = np.zeros((NSLOT, ED), F32)
        slot_bias = np.zeros(NSLOT, F32)

        emask = (dst >= s) & (dst < e)
        ce_src, ce_dst, ce_ea = src[emask], dst[emask], ea[emask]
        r_of_e = lid[ce_dst] // 128
        for r in range(NT):
            sel = np.where(r_of_e == r)[0]
            assert len(sel) <= CPR * 128, f"core {c} range {r}: {len(sel)}"
            base = r * CPR * 128
            sl = base + np.arange(len(sel))
            slot_pair[sl] = gpair[ce_src[sel]]
            slot_par[sl] = gpar[ce_src[sel]].astype(F32)
            slot_dst[sl] = lid[ce_dst[sel]] - r * 128
            slot_ea[sl] = ce_ea[sel]
            slot_bias[sl] = 1.0

        # host-precomputed one-hots (fp16): slot-major ssc for aggregation,
        # dst-major eoh (x16) for the dst-projection expansion
        ssc = np.zeros((128, NSLOT), F16)
        eoh = np.zeros((128, NSLOT), F16)
        sl_all = np.arange(NSLOT)
        ok = slot_dst < 128
        chunk_of = sl_all // 128
        pos_in = sl_all % 128
        ssc[pos_in[ok], chunk_of[ok] * 128 + slot_dst[ok]] = 1.0
        eoh[slot_dst[ok], sl_all[ok]] = 16.0

        # graph one-hot for pooling over local (permuted) node layout
        goh = np.zeros((128, NT * GPC), F32)
        xa_local = np.zeros(N_LOC, np.int64)
        nodes = np.arange(s, e)
        li = lid[nodes]
        xa_local[li] = x_atom[nodes]
        t_i, p_i = li // 128, li % 128
        goh[p_i, t_i * GPC + (batch[nodes] - c * GPC)] = 1.0

        goh2 = np.zeros((GPC, N_LOC), F32)
        goh2[batch[nodes] - c * GPC, li] = 1.0

        m = {
            "gidx": _wrap16(slot_pair, NSLOT),
            "xidx": _wrap16(xa_local, N_LOC),
            "pmask": np.repeat(
                slot_par.reshape(1, -1), 128, axis=0
            ).astype(np.uint8),
            "ssc": ssc,
            "eoh": eoh,
            "eaT": np.concatenate(
                [slot_ea.T, slot_bias.reshape(1, -1)], axis=0
            ).astype(F16),
            "goh": goh,
            "goh2": goh2,
            "maskbias": ((goh - 1.0) * 1e30).astype(F32),
        }
        in_maps.append(m)

    # shared parameters
    conv_Wf = np.asarray(inputs["conv_Wf"], np.float64)
    conv_Ws = np.asarray(inputs["conv_Ws"], np.float64)
    conv_bf = np.asarray(inputs["conv_bf"], np.float64)
    conv_bs = np.asarray(inputs["conv_bs"], np.float64)
    conv_bn = np.asarray(inputs["conv_bn"], F32)

    wsrc = np.concatenate(
        [
            np.concatenate([conv_Wf[l, H : 2 * H], conv_Ws[l, H : 2 * H]], 1)
            for l in range(L)
        ],
        axis=1,
    )  # [128, L*256]
    wdst = np.concatenate(
        [
            np.concatenate([conv_Wf[l, :H], conv_Ws[l, :H]], 1)
            for l in range(L)
        ],
        axis=1,
    )
    wdhi, wdlo = _hilo(wdst, WSC)
    wea = np.concatenate(
        [
            np.concatenate(
                [
                    np.concatenate([conv_Wf[l, 2 * H :], conv_Ws[l, 2 * H :]], 1),
                    np.concatenate([conv_bf[l], conv_bs[l]]).reshape(1, -1),
                ],
                axis=0,
            )
            for l in range(L)
        ],
        axis=1,
    )  # [51, L*256]
    convss = np.concatenate(
        [
            np.concatenate([_rep(sc), _rep(sh)], axis=1)
            for sc, sh in ((_bn_fold(conv_bn[l])) for l in range(L))
        ],
        axis=1,
    )  # [128, L*256]

    psc, psh = _bn_fold(np.asarray(inputs["proj_bn"], F32),
                        bias=np.asarray(inputs["proj_b"], F32))
    h1sc, h1sh = _bn_fold(np.asarray(inputs["head_bn1"], F32),
                          bias=np.asarray(inputs["head_b1"], F32))
    h2sc, h2sh = _bn_fold(np.asarray(inputs["head_bn2"], F32),
                          bias=np.asarray(inputs["head_b2"], F32))

    shared = {
        "emb": np.asarray(inputs["emb"], F16),
        "projW": np.asarray(inputs["proj_W"], F16),
        "projss": np.concatenate([_rep(psc), _rep(psh)], axis=1),
        "wsrc": (np.asarray(wsrc, np.float64) / HSC).astype(F16),
        "wdhi": wdhi,
        "wdlo": wdlo,
        "wea": np.asarray(wea, F16),
        "convss": convss,
        "gatew1": np.asarray(inputs["gate_W1"], F32),
        "gateb14": np.tile(_rep(np.asarray(inputs["gate_b1"], F32)), (1, 4)),
        "gatew2r4": np.tile(
            _rep(np.asarray(inputs["gate_W2"], F32).reshape(-1)), (1, 4)),
        "gateb2": _rep(np.asarray(inputs["gate_b2"], F32).reshape(1)),
        "projsc2": np.tile(_rep(psc), (1, 2)),
        "projsh2": np.tile(_rep(psh), (1, 2)),
        "headw1": np.asarray(inputs["head_W1"], F32),
        "h1ss": np.concatenate([_rep(h1sc), _rep(h1sh)], axis=1),
        "headw2": np.asarray(inputs["head_W2"], F32),
        "h2ss": np.concatenate([_rep(h2sc), _rep(h2sh)], axis=1),
        "headw3": np.asarray(inputs["head_W3"], F32),
        "h3b": _rep(np.asarray(inputs["head_b3"], F32)),
        "headw4": np.asarray(inputs["head_W4"], F32),
        "h4b": _rep(np.asarray(inputs["head_b4"], F32).reshape(1)),
        "identf": np.eye(128, dtype=F32),
        "identb": np.eye(128, dtype=F16),
    }
    for m in in_maps:
        m.update(shared)
    return in_maps


# ---------------------------------------------------------------------------
# bass program
# ---------------------------------------------------------------------------

def _build():
    dt = mybir.dt
    nc = bacc.Bacc(num_devices=C)

    def par(name, shape, dtp):
        return nc.declare_dram_parameter(name, list(shape), dtp, isOutput=False)

    gidx_d = par("gidx", [128, NSLOT // 16], dt.int16)
    xidx_d = par("xidx", [128, N_LOC // 16], dt.int16)
    pmask_d = par("pmask", [128, NSLOT], dt.uint8)
    ssc_d = par("ssc", [128, NSLOT], dt.float16)
    eoh_d = par("eoh", [128, NSLOT], dt.float16)
    eaT_d = par("eaT", [ED + 1, NSLOT], dt.float16)
    goh_d = par("goh", [128, NT * GPC], dt.float32)
    goh2_d = par("goh2", [GPC, N_LOC], dt.float32)
    maskbias_d = par("maskbias", [128, NT * GPC], dt.float32)
    emb_d = par("emb", [NEMB, H], dt.float16)
    projW_d = par("projW", [H, H], dt.float16)
    projss_d = par("projss", [128, 256], dt.float32)
    wsrc_d = par("wsrc", [H, L * 256], dt.float16)
    wdhi_d = par("wdhi", [H, L * 256], dt.float16)
    wdlo_d = par("wdlo", [H, L * 256], dt.float16)
    wea_d = par("wea", [ED + 1, L * 256], dt.float16)
    convss_d = par("convss", [128, L * 256], dt.float32)
    gatew1_d = par("gatew1", [H, H // 2], dt.float32)
    gateb14_d = par("gateb14", [128, 256], dt.float32)
    gatew2r4_d = par("gatew2r4", [128, 256], dt.float32)
    gateb2_d = par("gateb2", [128, 1], dt.float32)
    projsc2_d = par("projsc2", [128, 256], dt.float32)
    projsh2_d = par("projsh2", [128, 256], dt.float32)
    headw1_d = par("headw1", [H, H], dt.float32)
    h1ss_d = par("h1ss", [128, 256], dt.float32)
    headw2_d = par("headw2", [H, H // 2], dt.float32)
    h2ss_d = par("h2ss", [128, 128], dt.float32)
    headw3_d = par("headw3", [H // 2, H // 4], dt.float32)
    h3b_d = par("h3b", [128, H // 4], dt.float32)
    headw4_d = par("headw4", [H // 4, 1], dt.float32)
    h4b_d = par("h4b", [128, 1], dt.float32)
    identf_d = par("identf", [128, 128], dt.float32)
    identb_d = par("identb", [128, 128], dt.float16)

    out_d = nc.declare_dram_parameter("out", [GPC, 1], dt.float32, isOutput=True)

    hstage = nc.dram_tensor("hstage", [N_LOC // 2, 256], dt.float16)
    hfull = [
        nc.dram_tensor(f"hfull{i}", [PAIRS, 256], dt.float16,
                       addr_space="Shared")
        for i in range(2)
    ]

    FT, HT = dt.float32, dt.float16
    AF = mybir.ActivationFunctionType
    OP = mybir.AluOpType

    with tile.TileContext(nc) as tc:
        with (
            tc.tile_pool(name="const", bufs=1) as cpool,
            tc.tile_pool(name="state", bufs=1) as spool,
            tc.tile_pool(name="psA", bufs=3, space="PSUM") as psA,   # fs
            tc.tile_pool(name="psB", bufs=1, space="PSUM") as psB,   # p / head
            tc.tile_pool(name="psG", bufs=2, space="PSUM") as psG,   # aggr
        ):
            # ---------------- resident tiles ----------------
            def load(pool, dram, shape, dtp):
                nm = f"c_{dram.name}"
                t = pool.tile(shape, dtp, name=nm, tag=nm)
                nc.sync.dma_start(out=t[:], in_=dram[:])
                return t

            gidx_t = load(cpool, gidx_d, [128, NSLOT // 16], dt.int16)
            wsrc_t = load(cpool, wsrc_d, [H, L * 256], HT)
            wdhi_t = load(cpool, wdhi_d, [H, L * 256], HT)
            wdlo_t = load(cpool, wdlo_d, [H, L * 256], HT)
            wea_t = load(cpool, wea_d, [ED + 1, L * 256], HT)
            convss_t = load(cpool, convss_d, [128, L * 256], FT)
            identf_t = load(cpool, identf_d, [128, 128], FT)
            identb_t = load(cpool, identb_d, [128, 128], HT)

            h_loc = spool.tile([128, NT, H], FT, tag="h_loc")
            h16 = spool.tile([128, NT, H], HT, tag="h16")
            hres = spool.tile([128, NT, H], HT, tag="hres")

            dma_sem = nc.alloc_semaphore("swdge_dma")
            # manual gating for prepared gathers: Tile's deferred-dep path
            # gates neither triggers on the AllGather nor consumers on DMA
            # completion. Consumers wait on dma_sem (+16 per gather DMA,
            # descriptor-embedded). Each layer's triggers are gated on its
            # h table by a small PLAIN gather of hf (Tile makes it wait
            # for the collective; queue order covers the triggers behind).
            n_gather = [0]  # cumulative prepared gathers triggered

            def emit_stage(s, hf_next):
                # cast + stage + AllGather one range slice of the next
                # layer's h table (slices 0-2: 16 ranges, slice 3: 4).
                r0, nr = 16 * s, (16 if s < 3 else 4)
                hl = h_loc[:, r0 : r0 + nr, :].rearrange("p t h -> p (t h)")
                h16s = h16[:, r0 : r0 + nr, :].rearrange("p t h -> p (t h)")
                nc.vector.tensor_scalar_mul(out=h16s, in0=hl, scalar1=HSC)
                nc.vector.scalar_tensor_tensor(
                    out=hres[:, r0 : r0 + nr, :].rearrange("p t h -> p (t h)"),
                    in0=hl, scalar=HSC, in1=h16s,
                    op0=OP.mult, op1=OP.subtract)
                nc.sync.dma_start(
                    out=hstage[1024 * s : 1024 * s + 64 * nr, :]
                    .rearrange("n (two h) -> (n two) h", two=2)
                    .rearrange("(t p) h -> p t h", p=128),
                    in_=h16[:, r0 : r0 + nr, :],
                )
                nc.gpsimd.collective_compute(
                    "AllGather",
                    mybir.AluOpType.bypass,
                    replica_groups=[list(range(C))],
                    ins=[hstage[1024 * s : 1024 * s + 64 * nr, :]],
                    outs=[hf_next[8192 * s : 8192 * s + 512 * nr, :]],
                )

            # ---------------- embedding + projection ----------------
            with (
                tc.tile_pool(name="proj", bufs=2) as prpool,
                tc.tile_pool(name="projc", bufs=1) as prcpool,
                tc.tile_pool(name="psHp", bufs=1, space="PSUM") as psHp,
            ):
                xidx_t = load(prcpool, xidx_d, [128, N_LOC // 16], dt.int16)
                projW_t = load(prcpool, projW_d, [H, H], HT)
                projsc2_t = load(prcpool, projsc2_d, [128, 256], FT)
                projsh2_t = load(prcpool, projsh2_d, [128, 256], FT)
                # gather groups match the 16/16/16/4 staging slices so each
                # slice's cast+AllGather fires as soon as its tiles land
                t0g = 0
                for g, ntt in enumerate((16, 16, 16, 4)):
                    h0h = prpool.tile([128, 16, H], HT, tag="h0h",
                                      name=f"h0h{g}")
                    nc.gpsimd.dma_gather(
                        h0h[:, :ntt, :], emb_d[:],
                        xidx_t[:, t0g * 8 : (t0g + ntt) * 8],
                        ntt * 128, ntt * 128, H, single_packet=False,
                    )
                    for tt2 in range(0, ntt, 2):
                        t0 = t0g + tt2
                        pm = psB.tile([128, 256], FT, tag="pB",
                                      name=f"prm{t0}")
                        for k in range(2):
                            t = t0 + k
                            pT = psHp.tile([128, 128], HT, tag="trh",
                                           name=f"prT{t}")
                            nc.tensor.transpose(pT[:], h0h[:, tt2 + k, :],
                                                identb_t[:])
                            hT = prpool.tile([128, 128], HT, tag="hT16",
                                             name=f"prh{t}")
                            nc.vector.tensor_copy(hT[:], pT[:])
                            nc.tensor.matmul(pm[:, k * H : (k + 1) * H],
                                             hT[:], projW_t[:],
                                             start=True, stop=True)
                        t1 = prpool.tile([128, 256], FT, tag="nupd",
                                         name=f"pru{t0}")
                        nc.vector.tensor_tensor(
                            out=t1[:], in0=pm[:], in1=projsc2_t[:],
                            op=OP.mult)
                        nc.vector.tensor_tensor(
                            out=t1[:], in0=t1[:], in1=projsh2_t[:],
                            op=OP.add)
                        sgp = prpool.tile([128, 256], FT, tag="sgp",
                                          name=f"prs{t0}")
                        nc.scalar.activation(sgp[:], t1[:], AF.Sigmoid)
                        nc.vector.tensor_mul(
                            out=h_loc[:, t0 : t0 + 2, :]
                            .rearrange("p t h -> p (t h)"),
                            in0=t1[:], in1=sgp[:])
                    if _L_RUN > 0:
                        emit_stage(g, hfull[0])
                    t0g += ntt

            # ---------------- conv layers ----------------
            with (
                tc.tile_pool(name="gbuf", bufs=3) as gpool,
                tc.tile_pool(name="work", bufs=2) as wpool,
                tc.tile_pool(name="pq", bufs=1) as pqpool,
                tc.tile_pool(name="mb", bufs=2) as mbpool,
                tc.tile_pool(name="mb1", bufs=1) as mb1pool,
                tc.tile_pool(name="uq", bufs=1) as uqpool,
                tc.tile_pool(name="psHc", bufs=2, space="PSUM") as psHc,
            ):
                for l in range(_L_RUN):
                    hf = hfull[l % 2]
                    lsl = slice(l * 256, (l + 1) * 256)
                    # h16/hres for this layer were cast slice-by-slice
                    # during the previous layer's block loop (or the
                    # projection phase for l=0); the AllGathers are in
                    # flight already.
                    aggr = {}
                    # dst-side projections p = h @ Wdst in ~fp32:
                    # (h16+hres) @ (wdhi+wdlo), hres@wdlo dropped.
                    # Emitted before the collective so the PE chews on them
                    # during the AllGather.
                    p_hi = pqpool.tile([128, NT, 256], HT, tag="p_hi",
                                       name=f"p_hi_{l}")
                    p_lo = pqpool.tile([128, NT, 256], HT, tag="p_lo",
                                       name=f"p_lo_{l}")
                    for t in range(NT):
                        pT = psHc.tile([128, 128], HT, tag="trh",
                                       name=f"pT_{l}_{t}")
                        nc.tensor.transpose(pT[:], h16[:, t, :],
                                            identb_t[:])
                        hTb = wpool.tile([128, 128], HT, tag="hTb",
                                         name=f"hTb_{l}_{t}")
                        nc.scalar.activation(hTb[:], pT[:], AF.Identity)
                        pT2 = psHc.tile([128, 128], HT, tag="trh",
                                        name=f"pT2_{l}_{t}")
                        nc.tensor.transpose(pT2[:], hres[:, t, :],
                                            identb_t[:])
                        hTr = wpool.tile([128, 128], HT, tag="hTr",
                                         name=f"hTr_{l}_{t}")
                        nc.vector.tensor_copy(hTr[:], pT2[:])
                        pm = psB.tile([128, 256], FT, tag="pB",
                                      name=f"pm_{l}_{t}")
                        nc.tensor.matmul(pm[:], hTb[:], wdhi_t[:, lsl],
                                         start=True, stop=False)
                        nc.tensor.matmul(pm[:], hTb[:], wdlo_t[:, lsl],
                                         start=False, stop=False)
                        nc.tensor.matmul(pm[:], hTr[:], wdhi_t[:, lsl],
                                         start=False, stop=True)
                        nc.scalar.activation(p_hi[:, t, :], pm[:],
                                             AF.Identity, scale=1.0 / WSC)
                        if _PLO:
                            nc.vector.scalar_tensor_tensor(
                                out=p_lo[:, t, :], in0=pm[:],
                                scalar=1.0 / WSC, in1=p_hi[:, t, :],
                                op0=OP.mult, op1=OP.subtract)

                    pend = None  # software-pipelined aggregation

                    def emit_aggr(pb, pmsg, pssc, l=l):
                        for j in range(CPB):
                            c = pb * CPB + j
                            r = c // CPR
                            qd = r // 4
                            sl = slice(j * 128, (j + 1) * 128)
                            if c % 16 == 0:
                                aggr[qd] = psG.tile(
                                    [128, 4 * H], FT, tag="aggr",
                                    name=f"aggr_{l}_{qd}")
                            osl = slice((r % 4) * H, (r % 4 + 1) * H)
                            nc.tensor.matmul(
                                aggr[qd][:, osl], pssc[:, sl],
                                pmsg[:, j, :],
                                start=(c % CPR == 0),
                                stop=(c % CPR == CPR - 1))
                            if c % 16 == 15:
                                # quad-batched BN + silu + residual
                                r0 = qd * 4
                                u4 = uqpool.tile(
                                    [128, 4, H], FT, tag="u4",
                                    name=f"u_{l}_{qd}")
                                u4f = u4[:].rearrange("p c h -> p (c h)")
                                h4 = h_loc[:, r0 : r0 + 4, :].rearrange(
                                    "p c h -> p (c h)")
                                nc.vector.tensor_tensor(
                                    out=u4f, in0=aggr[qd][:], in1=h4,
                                    op=OP.add)
                                del aggr[qd]
                                for i in range(4):
                                    nc.vector.tensor_tensor(
                                        out=u4[:, i, :], in0=u4[:, i, :],
                                        in1=convss_t[
                                            :, l * 256 : l * 256 + 128],
                                        op=OP.mult)
                                    nc.vector.tensor_tensor(
                                        out=u4[:, i, :], in0=u4[:, i, :],
                                        in1=convss_t[
                                            :, l * 256 + 128
                                            : (l + 1) * 256],
                                        op=OP.add)
                                us = uqpool.tile(
                                    [128, 4 * H], FT, tag="us",
                                    name=f"us_{l}_{qd}")
                                nc.scalar.activation(us[:], u4f,
                                                     AF.Sigmoid)
                                nc.vector.tensor_mul(out=u4f, in0=u4f,
                                                     in1=us[:])
                                nc.vector.tensor_tensor(
                                    out=h4, in0=h4, in1=u4f, op=OP.add)
                                # next layer's table: stage slice s as
                                # soon as its last qd group (4s+3) lands
                                if l + 1 < _L_RUN and qd in (3, 7, 11):
                                    emit_stage(qd // 4, hfull[(l + 1) % 2])

                    for q in range(4):  # quarters: 13 ranges, 4 blocks each
                        for bq in range(4):
                            b = q * 4 + bq
                            bsl = slice(b * SLOT_B, (b + 1) * SLOT_B)
                            if _PREP and b == 0:
                                # AG fence: a plain 128-slot gather of hf
                                # blocks gpsimd until the table's
                                # AllGathers land (Tile-managed dep).
                                # Must precede the prep: plain descriptors
                                # cannot pass an untriggered ring entry.
                                fence = wpool.tile(
                                    [128, 1, 256], HT, tag="agfence",
                                    name=f"fence_{l}")
                                nc.gpsimd.dma_gather(
                                    fence[:], hf[:], gidx_t[:, :8],
                                    128, 128, 256,
                                    single_packet=False)
                            gb = gpool.tile([128, 2, SLOT_B], HT, tag="gb",
                                            name=f"gb_{l}_{b}")
                            nc.gpsimd.dma_gather(
                                gb[:], hf[:],
                                gidx_t[:, b * (SLOT_B // 16) : (b + 1) * (SLOT_B // 16)],
                                SLOT_B, SLOT_B, 256, transpose=True,
                                single_packet=False,
                                prepare_only=_PREP, sem=dma_sem if _PREP else None,
                            )
                            if _PREP:
                                nc.gpsimd.trigger_dma(count=None)
                                n_gather[0] += 1
                            mask = wpool.tile([128, SLOT_B], dt.uint8,
                                              tag="mask", name=f"mk_{l}_{b}")
                            nc.sync.dma_start(out=mask[:], in_=pmask_d[:, bsl])
                            ea_t = wpool.tile([ED + 1, SLOT_B], HT, tag="ea",
                                              name=f"ea_{l}_{b}")
                            nc.sync.dma_start(out=ea_t[:], in_=eaT_d[:, bsl])
                            ssc_t = wpool.tile([128, SLOT_B], HT, tag="sscb",
                                               name=f"ssc_{l}_{b}")
                            nc.sync.dma_start(out=ssc_t[:], in_=ssc_d[:, bsl])
                            eoh_t = wpool.tile([128, SLOT_B], HT, tag="eohb",
                                               name=f"eoh_{l}_{b}")
                            nc.sync.dma_start(out=eoh_t[:], in_=eoh_d[:, bsl])
                            # parity select in place: gb[:,0,:] becomes the
                            # merged h_src plane. With bufs=3 the next
                            # gather's WAR on this buf (now extended to the
                            # chunk matmul reads) keeps a 3-block lead.
                            nc.vector.copy_predicated(gb[:, 0, :], mask[:],
                                                      gb[:, 1, :])

                            sg_b = mbpool.tile([128, CPB, H], HT, tag="sg_b",
                                               name=f"sg_{l}_{b}")
                            s16_b = mbpool.tile([128, CPB, H], HT, tag="s16",
                                                name=f"s16_{l}_{b}")
                            for j in range(CPB):
                                c = b * CPB + j
                                r = c // CPR
                                sl = slice(j * 128, (j + 1) * 128)
                                fs = psA.tile([128, 256], FT, tag="fs",
                                              name=f"fs_{l}_{c}")
                                nc.tensor.matmul(
                                    fs[:], gb[:, 0, sl], wsrc_t[:, lsl],
                                    start=True, stop=False)
                                nc.tensor.matmul(
                                    fs[:], ea_t[:, sl], wea_t[:, lsl],
                                    start=False, stop=False)
                                nc.tensor.matmul(
                                    fs[:], eoh_t[:, sl], p_hi[:, r, :],
                                    start=False, stop=not _PLO)
                                if _PLO:
                                    nc.tensor.matmul(
                                        fs[:], eoh_t[:, sl], p_lo[:, r, :],
                                        start=False, stop=True)
                                nc.scalar.activation(sg_b[:, j, :], fs[:, :H],
                                                     AF.Sigmoid)
                                nc.vector.tensor_copy(s16_b[:, j, :],
                                                      fs[:, H:])

                            if pend is not None:
                                emit_aggr(*pend)
                                pend = None

                            # block-batched softplus:
                            # sp = max(s, ln(1+exp(min(s,10))))
                            s10 = mb1pool.tile([128, CPB * H], HT, tag="s10",
                                              name=f"s10_{l}_{b}")
                            nc.vector.tensor_scalar_min(
                                out=s10[:],
                                in0=s16_b[:].rearrange("p c h -> p (c h)"),
                                scalar1=10.0)
                            e_b = mb1pool.tile([128, CPB * H], HT, tag="e_b",
                                              name=f"e_{l}_{b}")
                            nc.scalar.activation(e_b[:], s10[:], AF.Exp)
                            sp0 = mb1pool.tile([128, CPB * H], HT, tag="sp0",
                                              name=f"sp0_{l}_{b}")
                            nc.scalar.activation(sp0[:], e_b[:], AF.Ln,
                                                 bias=1.0)
                            spm = mb1pool.tile([128, CPB * H], HT, tag="spm",
                                              name=f"spm_{l}_{b}")
                            nc.vector.tensor_max(
                                out=spm[:], in0=sp0[:],
                                in1=s16_b[:].rearrange("p c h -> p (c h)"))
                            msg = mbpool.tile([128, CPB, H], HT, tag="msg",
                                              name=f"msg_{l}_{b}")
                            nc.vector.tensor_mul(
                                out=msg[:].rearrange("p c h -> p (c h)"),
                                in0=sg_b[:].rearrange("p c h -> p (c h)"),
                                in1=spm[:])

                            pend = (b, msg, ssc_t)

                    emit_aggr(*pend)
                    pend = None
                    if l + 1 < _L_RUN:
                        emit_stage(3, hfull[(l + 1) % 2])

            # ---------------- gate + pooling + head ----------------
            with (
                tc.tile_pool(name="poolc", bufs=1) as pcpool,
                tc.tile_pool(name="pools", bufs=3) as smpool,
                tc.tile_pool(name="psP", bufs=2, space="PSUM") as psP,
            ):
                goh_t = load(pcpool, goh_d, [128, NT * GPC], FT)
                goh2_t = load(pcpool, goh2_d, [GPC, N_LOC], FT)
                maskb_t = load(pcpool, maskbias_d, [128, NT * GPC], FT)
                gatew1_t = load(pcpool, gatew1_d, [H, H // 2], FT)
                gateb14_t = load(pcpool, gateb14_d, [128, 256], FT)
                gatew2r4_t = load(pcpool, gatew2r4_d, [128, 256], FT)
                gateb2_t = load(pcpool, gateb2_d, [128, 1], FT)
                headw1_t = load(pcpool, headw1_d, [H, H], FT)
                h1ss_t = load(pcpool, h1ss_d, [128, 256], FT)
                headw2_t = load(pcpool, headw2_d, [H, H // 2], FT)
                h2ss_t = load(pcpool, h2ss_d, [128, 128], FT)
                headw3_t = load(pcpool, headw3_d, [H // 2, H // 4], FT)
                h3b_t = load(pcpool, h3b_d, [128, H // 4], FT)
                headw4_t = load(pcpool, headw4_d, [H // 4, 1], FT)
                h4b_t = load(pcpool, h4b_d, [128, 1], FT)

                g_all = pcpool.tile([128, NT], FT, name="g_all", tag="g_all")
                runmax = pcpool.tile([128, GPC], FT, name="runmax",
                                     tag="runmax")

                # pass 1: gate scores g, 4 tiles per round; the per-graph
                # max falls out of one masked [128, NT*GPC] array at the end
                gmx = pcpool.tile([128, NT * GPC], FT, name="gmx", tag="gmx")
                for gq in range(NT // 4):
                    g1 = psB.tile([128, 256], FT, tag="pB", name=f"g1_{gq}")
                    for i in range(4):
                        t = 4 * gq + i
                        pT = psP.tile([128, 128], FT, tag="tr", name=f"gT{t}")
                        nc.tensor.transpose(pT[:], h_loc[:, t, :],
                                            identf_t[:])
                        hT = smpool.tile([128, 128], FT, tag="hT32",
                                         name=f"gh{t}")
                        nc.vector.tensor_copy(hT[:], pT[:])
                        nc.tensor.matmul(g1[:, i * 64 : i * 64 + 64], hT[:],
                                         gatew1_t[:], start=True, stop=True)
                    s1 = smpool.tile([128, 256], FT, tag="s1",
                                     name=f"s1_{gq}")
                    nc.vector.tensor_tensor(out=s1[:], in0=g1[:],
                                            in1=gateb14_t[:], op=OP.add)
                    s1s = smpool.tile([128, 256], FT, tag="s1s",
                                      name=f"s1s_{gq}")
                    nc.scalar.activation(s1s[:], s1[:], AF.Silu)
                    nc.vector.tensor_mul(out=s1s[:], in0=s1s[:],
                                         in1=gatew2r4_t[:])
                    nc.vector.tensor_reduce(
                        out=g_all[:, 4 * gq : 4 * gq + 4]
                        .rearrange("p (t o) -> p t o", o=1),
                        in_=s1s[:].rearrange("p (t k) -> p t k", t=4),
                        axis=mybir.AxisListType.X, op=OP.add)
                    nc.vector.tensor_scalar(
                        out=g_all[:, 4 * gq : 4 * gq + 4],
                        in0=g_all[:, 4 * gq : 4 * gq + 4],
                        scalar1=gateb2_t[:], scalar2=None, op0=OP.add)
                nc.vector.tensor_tensor(
                    out=gmx[:].rearrange("p (t k) -> p t k", k=GPC),
                    in0=g_all[:].rearrange("p (t o) -> p t o", o=1)
                    .to_broadcast([128, NT, GPC]),
                    in1=goh_t[:].rearrange("p (t k) -> p t k", k=GPC),
                    op=OP.mult)
                nc.vector.tensor_tensor(out=gmx[:], in0=gmx[:],
                                        in1=maskb_t[:], op=OP.add)
                nc.vector.tensor_reduce(
                    out=runmax[:],
                    in_=gmx[:].rearrange("p (t k) -> p k t", k=GPC),
                    axis=mybir.AxisListType.X, op=OP.max)

                # reduce running max across partitions -> gmax [GPC, 1]
                pTm = psP.tile([128, 128], FT, tag="tr", name="pTm")
                nc.tensor.transpose(pTm[:GPC, :], runmax[:], identf_t[:])
                rmT = smpool.tile([GPC, 128], FT, tag="rmT", name="rmT")
                nc.vector.tensor_copy(rmT[:], pTm[:GPC, :])
                negmax = smpool.tile([GPC, 1], FT, tag="negmax",
                                     name="negmax")
                nc.vector.tensor_reduce(out=negmax[:], in_=rmT[:],
                                        axis=mybir.AxisListType.X,
                                        op=OP.max)
                nc.vector.tensor_scalar_mul(out=negmax[:], in0=negmax[:],
                                            scalar1=-1.0)

                # pass 2: e = exp(min(g - gmax[graph], 20)) batched across
                # all tiles, then the pooled-sum matmuls
                eps_ps = psB.tile([128, 256], FT, tag="pB", name="eps")
                for t in range(NT):
                    nc.tensor.matmul(
                        eps_ps[:, t : t + 1],
                        goh2_t[:, t * 128 : (t + 1) * 128],
                        negmax[:], start=True, stop=True)
                earg = smpool.tile([128, NT], FT, tag="earg", name="earg")
                nc.vector.tensor_tensor(out=earg[:], in0=g_all[:],
                                        in1=eps_ps[:, :NT], op=OP.add)
                nc.vector.tensor_scalar_min(out=earg[:], in0=earg[:],
                                            scalar1=20.0)
                e_all = smpool.tile([128, NT], FT, tag="ecol", name="e_all")
                nc.scalar.activation(e_all[:], earg[:], AF.Exp)
                pool_ps = psA.tile([GPC, H + 1], FT, tag="fs", name="pool_ps")
                for t in range(NT):
                    rhs = smpool.tile([128, H + 1], FT, tag="rhs",
                                      name=f"rhs_{t}")
                    nc.vector.tensor_scalar(
                        out=rhs[:, :H], in0=h_loc[:, t, :],
                        scalar1=e_all[:, t : t + 1],
                        scalar2=None, op0=OP.mult)
                    nc.vector.tensor_copy(rhs[:, H : H + 1],
                                          e_all[:, t : t + 1])
                    nc.tensor.matmul(
                        pool_ps[:], goh_t[:, t * GPC : (t + 1) * GPC], rhs[:],
                        start=(t == 0), stop=(t == NT - 1))

                pooled_raw = smpool.tile([GPC, H + 1], FT, tag="praw")
                nc.vector.tensor_copy(pooled_raw[:], pool_ps[:])
                rec = smpool.tile([GPC, 1], FT, tag="rec")
                nc.vector.reciprocal(rec[:], pooled_raw[:, H : H + 1])
                pooled = smpool.tile([GPC, H], FT, tag="pooled")
                nc.vector.tensor_scalar(
                    out=pooled[:], in0=pooled_raw[:, :H], scalar1=rec[:],
                    scalar2=None, op0=OP.mult)

                def head_mm(x, w, nin, nout, nm, ss=None, badd=None,
                            silu=True):
                    pT = psP.tile([128, 128], FT, tag="tr",
                                  name=f"hT{nm}")
                    nc.tensor.transpose(pT[:nin, :GPC], x[:],
                                        identf_t[:GPC, :GPC])
                    xT = smpool.tile([128, GPC], FT, tag="xT",
                                     name=f"xT{nm}")
                    nc.vector.tensor_copy(xT[:nin, :], pT[:nin, :GPC])
                    ym = psB.tile([128, 256], FT, tag="pB", name=f"ym{nm}")
                    nc.tensor.matmul(ym[:GPC, :nout], xT[:nin, :], w[:],
                                     start=True, stop=True)
                    y = smpool.tile([GPC, nout], FT, tag=f"hd{nout}",
                                    name=f"y{nm}")
                    if ss is not None:
                        nc.vector.tensor_tensor(
                            out=y[:], in0=ym[:GPC, :nout],
                            in1=ss[:GPC, :nout], op=OP.mult)
                        nc.vector.tensor_tensor(
                            out=y[:], in0=y[:], in1=ss[:GPC, nout : 2 * nout],
                            op=OP.add)
                    elif badd is not None:
                        nc.vector.tensor_tensor(
                            out=y[:], in0=ym[:GPC, :nout],
                            in1=badd[:GPC, :nout], op=OP.add)
                    else:
                        nc.vector.tensor_copy(y[:], ym[:GPC, :nout])
                    if silu:
                        ysig = smpool.tile([GPC, nout], FT,
                                           tag=f"hs{nout}", name=f"ys{nm}")
                        nc.scalar.activation(ysig[:], y[:], AF.Sigmoid)
                        nc.vector.tensor_mul(out=y[:], in0=y[:], in1=ysig[:])
                    return y

                y1 = head_mm(pooled, headw1_t, H, H, "a", ss=h1ss_t)
                y2 = head_mm(y1, headw2_t, H, H // 2, "b", ss=h2ss_t)
                y3 = head_mm(y2, headw3_t, H // 2, H // 4, "c", badd=h3b_t)
                y4 = head_mm(y3, headw4_t, H // 4, 1, "d", badd=h4b_t,
                             silu=False)
                nc.sync.dma_start(out=out_d[:], in_=y4[:])

    return nc


_NC_CACHE = None
_LAST_EXEC_NS = None


def kernel(**inputs) -> np.ndarray:
    global _NC_CACHE, _LAST_EXEC_NS
    in_maps = _prep(inputs)
    if _NC_CACHE is None:
        _NC_CACHE = _build()
        _NC_CACHE.finalize()
    trace = os.environ.get("KERNEL_TRACE", "0") == "1"
    res = run_bass_kernel_spmd(
        _NC_CACHE, in_maps, core_ids=list(range(C)), trace=trace,
        tmpdir=os.environ.get("KERNEL_TRACE_DIR") if trace else None,
    )
    _LAST_EXEC_NS = res.exec_time_ns
    out = np.concatenate(
        [np.asarray(res.results[c]["out"]).reshape(GPC) for c in range(C)]
    )
    return out.astype(F32)


if __name__ == "__main__":
    import jax

    with jax.default_device(jax.devices("cpu")[0]):
        sys.path.insert(0, os.path.dirname(os.path.abspath(__file__)))
        import reference

        inp = {k: np.asarray(v) for k, v in reference.setup_inputs().items()}
    y = kernel(**inp)
    print("out[:8]:", y[:8])



# revision 68
# speedup vs baseline: 1.1993x; 1.1993x over previous
"""CGCNN regressor on 8 trn2 NeuronCores.

Sharding: graphs 32/core -> contiguous node blocks; edges live on dst's core.
Per core, nodes are permuted into 52 "ranges" of 128 (degree-balanced bin
packing, <=512 edges/range); each range owns 4 edge chunks of 128 slots.
Per layer: h (fp16, x1/16) is AllGathered to a replicated pair-table
[26624, 256]; h[src] is fetched with one dma_gather(transpose=True) per
block; messages are fp16 matmuls (src + edge_attr + hi/lo dst-projection
expansion via host-precomputed one-hots) accumulating in PSUM. The dst
projection keeps ~fp32 precision via two fp16 planes of h and of Wdst.
sigmoid/softplus run block-batched (softplus = max(s, ln(1+exp(min(s,10))))
from the natural_log_exp table set). Aggregation is a one-hot fp16 matmul;
BN/silu/residual updates are quarter-batched. Pool/head run on 32
graphs/core; host concatenates the 8x[32] outputs.

Latency pipelining: the h table uses a slice-major pair numbering (range
slices [0:16)/[16:32)/[32:48)/[48:52) per core), so the next layer's
cast+stage+AllGather is emitted per-slice inside THIS layer's block loop
as soon as the slice's residual updates land; only the tiny 4-range tail
AllGather is exposed at the layer boundary. The projection phase gathers
the (host-precast fp16) embedding in the same 16/16/16/4 groups and
stages layer 0's table as each group lands. Gate pooling batches 4 node
tiles per round and derives the per-graph max from one masked
[128, NT*GPC] array; pass 2 computes all exp() in one activation call.
(prepare_only/trigger_dma gathers and HWDGE indirect gathers were tried
and abandoned: desc-gen dominates either way on this ucode, Tile's
deferred-dep path doesn't gate triggers on the collective, and
qPoolDynamic indirect DMA returns garbage on this runtime.)
"""

import os
import sys

import numpy as np

try:
    import concourse.bass as bass
except ImportError:  # grading env fallback
    sys.path.insert(0, "/opt/trn_rl_repo")
    import concourse.bass as bass

import concourse.mybir as mybir
import concourse.tile as tile
from concourse import bacc
from concourse.bass_utils import run_bass_kernel_spmd

F32 = np.float32
F16 = np.float16

# problem constants
N, E, H, ED, NG, NEMB, L = 50000, 200000, 128, 50, 256, 100, 6
C = 8               # cores
GPC = NG // C       # graphs per core
NT = 52             # node tiles (ranges) per core
N_LOC = NT * 128    # padded local nodes (6656)
CPR = 4             # chunks per range
NCHUNK = NT * CPR   # 208
NSLOT = NCHUNK * 128  # 26624 edge slots
CPB = 13            # chunks per gather block
NBLK = NCHUNK // CPB  # 8
SLOT_B = CPB * 128  # 1664 slots per block
RPQ = 13            # ranges per quarter
PAIRS = C * N_LOC // 2  # 26624 pair rows in the replicated h table
PAD_DST = 255       # dst sentinel for dummy slots (matches no one-hot row)
HSC = 1.0 / 16.0    # fp16 h-table scale (h stored as h*HSC; 16x in Wsrc)
WSC = 32.0          # wdst hi/lo plane scale (undone in the p evacuation)

_L_RUN = int(os.environ.get("KERNEL_LAYERS", str(L)))
_PREP = os.environ.get("KERNEL_PREP", "0") == "1"
_PLO = os.environ.get("KERNEL_PLO", "1") == "1"


# ---------------------------------------------------------------------------
# host-side preprocessing
# ---------------------------------------------------------------------------

def _wrap16(idx, pad_to):
    """int16 index tensor in dma_gather layout: [128, pad_to//16],
    slot i -> row i%16, col i//16; replicated 8x down the partitions."""
    a = np.full(pad_to, 0, np.int16)
    a[: len(idx)] = idx.astype(np.int16)
    w = a.reshape(pad_to // 16, 16).T  # [16, pad/16]
    return np.tile(w, (8, 1)).copy()


def _bn_fold(p, bias=None):
    gamma, beta, mean, var = [np.asarray(x, np.float64) for x in p]
    scale = gamma / np.sqrt(var + 1e-5)
    shift = beta - mean * scale
    if bias is not None:
        shift = shift + np.asarray(bias, np.float64) * scale
    return scale.astype(F32), shift.astype(F32)


def _rep(row, parts=128):
    row = np.asarray(row, F32).reshape(1, -1)
    return np.repeat(row, parts, axis=0).copy()


def _hilo(w, scale=1.0):
    w = np.asarray(w, np.float64) * scale
    hi = w.astype(F16)
    lo = (w - hi.astype(np.float64)).astype(F16)
    return hi, lo


def _prep(inputs):
    x_atom = np.asarray(inputs["x_atom"]).astype(np.int64)
    ei = np.asarray(inputs["edge_index"]).astype(np.int64)
    ea = np.asarray(inputs["edge_attr"]).astype(F32)
    batch = np.asarray(inputs["batch"]).astype(np.int64)
    src, dst = ei[0], ei[1]

    node_start = np.searchsorted(batch, np.arange(0, NG + 1, GPC))
    deg = np.bincount(dst, minlength=N)

    # global node -> (core, local id); degree-balanced FFD into NT ranges/core
    lid = np.empty(N, np.int64)
    core_of = np.empty(N, np.int64)
    for c in range(C):
        s, e = node_start[c], node_start[c + 1]
        nodes = np.arange(s, e)
        assert len(nodes) <= N_LOC, f"core {c}: {len(nodes)} > {N_LOC}"
        order = nodes[np.argsort(-deg[nodes], kind="stable")]
        cap_n = np.full(NT, 128, np.int64)
        cap_e = np.full(NT, CPR * 128, np.int64)
        pos = np.zeros(NT, np.int64)
        for g in order:
            d = deg[g]
            cand = np.where((cap_n > 0) & (cap_e >= d))[0]
            assert len(cand), f"core {c}: range packing failed (deg {d})"
            r = cand[np.argmax(cap_e[cand])]
            lid[g] = r * 128 + pos[r]
            pos[r] += 1
            cap_n[r] -= 1
            cap_e[r] -= d
        core_of[s:e] = c

    # slice-major pair address in the replicated table: range slices of
    # 16/16/16/4 ranges AllGather independently, so slice s of core c lands
    # at rows [8192*s + c*rows_s, ...) of hfull (rows_s = 1024 or 256).
    lid_r, lid_p = lid // 128, lid % 128
    sl_of = np.minimum(lid_r // 16, 3)
    rows_s = np.where(sl_of < 3, 1024, 256)
    gpair = sl_of * 8192 + core_of * rows_s + (lid_r - 16 * sl_of) * 64 \
        + lid_p // 2
    gpar = lid_p & 1

    in_maps = []
    for c in range(C):
        s, e = node_start[c], node_start[c + 1]
        slot_pair = np.zeros(NSLOT, np.int64)
        slot_par = np.zeros(NSLOT, F32)
        slot_dst = np.full(NSLOT, PAD_DST, np.int64)
        slot_ea = np.zeros((NSLOT, ED), F32)
        slot_bias = np.zeros(NSLOT, F32)

        emask = (dst >= s) & (dst < e)
        ce_src, ce_dst, ce_ea = src[emask], dst[emask], ea[emask]
        r_of_e = lid[ce_dst] // 128
        for r in range(NT):
            sel = np.where(r_of_e == r)[0]
            assert len(sel) <= CPR * 128, f"core {c} range {r}: {len(sel)}"
            base = r * CPR * 128
            sl = base + np.arange(len(sel))
            slot_pair[sl] = gpair[ce_src[sel]]
            slot_par[sl] = gpar[ce_src[sel]].astype(F32)
            slot_dst[sl] = lid[ce_dst[sel]] - r * 128
            slot_ea[sl] = ce_ea[sel]
            slot_bias[sl] = 1.0

        # host-precomputed one-hots (fp16): slot-major ssc for aggregation,
        # dst-major eoh (x16) for the dst-projection expansion
        ssc = np.zeros((128, NSLOT), F16)
        eoh = np.zeros((128, NSLOT), F16)
        sl_all = np.arange(NSLOT)
        ok = slot_dst < 128
        chunk_of = sl_all // 128
        pos_in = sl_all % 128
        ssc[pos_in[ok], chunk_of[ok] * 128 + slot_dst[ok]] = 1.0
        eoh[slot_dst[ok], sl_all[ok]] = 16.0

        # graph one-hot for pooling over local (permuted) node layout
        goh = np.zeros((128, NT * GPC), F32)
        xa_local = np.zeros(N_LOC, np.int64)
        nodes = np.arange(s, e)
        li = lid[nodes]
        xa_local[li] = x_atom[nodes]
        t_i, p_i = li // 128, li % 128
        goh[p_i, t_i * GPC + (batch[nodes] - c * GPC)] = 1.0

        goh2 = np.zeros((GPC, N_LOC), F32)
        goh2[batch[nodes] - c * GPC, li] = 1.0

        m = {
            "gidx": _wrap16(slot_pair, NSLOT),
            "xidx": _wrap16(xa_local, N_LOC),
            "pmask": np.repeat(
                slot_par.reshape(1, -1), 128, axis=0
            ).astype(np.uint8),
            "ssc": ssc,
            "eoh": eoh,
            "eaT": np.concatenate(
                [slot_ea.T, slot_bias.reshape(1, -1)], axis=0
            ).astype(F16),
            "goh": goh,
            "goh2": goh2,
            "maskbias": ((goh - 1.0) * 1e30).astype(F32),
        }
        in_maps.append(m)

    # shared parameters
    conv_Wf = np.asarray(inputs["conv_Wf"], np.float64)
    conv_Ws = np.asarray(inputs["conv_Ws"], np.float64)
    conv_bf = np.asarray(inputs["conv_bf"], np.float64)
    conv_bs = np.asarray(inputs["conv_bs"], np.float64)
    conv_bn = np.asarray(inputs["conv_bn"], F32)

    wsrc = np.concatenate(
        [
            np.concatenate([conv_Wf[l, H : 2 * H], conv_Ws[l, H : 2 * H]], 1)
            for l in range(L)
        ],
        axis=1,
    )  # [128, L*256]
    wdst = np.concatenate(
        [
            np.concatenate([conv_Wf[l, :H], conv_Ws[l, :H]], 1)
            for l in range(L)
        ],
        axis=1,
    )
    wdhi, wdlo = _hilo(wdst, WSC)
    wea = np.concatenate(
        [
            np.concatenate(
                [
                    np.concatenate([conv_Wf[l, 2 * H :], conv_Ws[l, 2 * H :]], 1),
                    np.concatenate([conv_bf[l], conv_bs[l]]).reshape(1, -1),
                ],
                axis=0,
            )
            for l in range(L)
        ],
        axis=1,
    )  # [51, L*256]
    convss = np.concatenate(
        [
            np.concatenate([_rep(sc), _rep(sh)], axis=1)
            for sc, sh in ((_bn_fold(conv_bn[l])) for l in range(L))
        ],
        axis=1,
    )  # [128, L*256]

    psc, psh = _bn_fold(np.asarray(inputs["proj_bn"], F32),
                        bias=np.asarray(inputs["proj_b"], F32))
    h1sc, h1sh = _bn_fold(np.asarray(inputs["head_bn1"], F32),
                          bias=np.asarray(inputs["head_b1"], F32))
    h2sc, h2sh = _bn_fold(np.asarray(inputs["head_bn2"], F32),
                          bias=np.asarray(inputs["head_b2"], F32))

    shared = {
        "emb": np.asarray(inputs["emb"], F16),
        "projW": np.asarray(inputs["proj_W"], F16),
        "projss": np.concatenate([_rep(psc), _rep(psh)], axis=1),
        "wsrc": (np.asarray(wsrc, np.float64) / HSC).astype(F16),
        "wdhi": wdhi,
        "wdlo": wdlo,
        "wea": np.asarray(wea, F16),
        "convss": convss,
        "gatew1": np.asarray(inputs["gate_W1"], F32),
        "gateb14": np.tile(_rep(np.asarray(inputs["gate_b1"], F32)), (1, 4)),
        "gatew2r4": np.tile(
            _rep(np.asarray(inputs["gate_W2"], F32).reshape(-1)), (1, 4)),
        "gateb2": _rep(np.asarray(inputs["gate_b2"], F32).reshape(1)),
        "projsc2": np.tile(_rep(psc), (1, 2)),
        "projsh2": np.tile(_rep(psh), (1, 2)),
        "headw1": np.asarray(inputs["head_W1"], F32),
        "h1ss": np.concatenate([_rep(h1sc), _rep(h1sh)], axis=1),
        "headw2": np.asarray(inputs["head_W2"], F32),
        "h2ss": np.concatenate([_rep(h2sc), _rep(h2sh)], axis=1),
        "headw3": np.asarray(inputs["head_W3"], F32),
        "h3b": _rep(np.asarray(inputs["head_b3"], F32)),
        "headw4": np.asarray(inputs["head_W4"], F32),
        "h4b": _rep(np.asarray(inputs["head_b4"], F32).reshape(1)),
        "identf": np.eye(128, dtype=F32),
        "identb": np.eye(128, dtype=F16),
    }
    for m in in_maps:
        m.update(shared)
    return in_maps


# ---------------------------------------------------------------------------
# bass program
# ---------------------------------------------------------------------------

def _build():
    dt = mybir.dt
    nc = bacc.Bacc(num_devices=C)

    def par(name, shape, dtp):
        return nc.declare_dram_parameter(name, list(shape), dtp, isOutput=False)

    gidx_d = par("gidx", [128, NSLOT // 16], dt.int16)
    xidx_d = par("xidx", [128, N_LOC // 16], dt.int16)
    pmask_d = par("pmask", [128, NSLOT], dt.uint8)
    ssc_d = par("ssc", [128, NSLOT], dt.float16)
    eoh_d = par("eoh", [128, NSLOT], dt.float16)
    eaT_d = par("eaT", [ED + 1, NSLOT], dt.float16)
    goh_d = par("goh", [128, NT * GPC], dt.float32)
    goh2_d = par("goh2", [GPC, N_LOC], dt.float32)
    maskbias_d = par("maskbias", [128, NT * GPC], dt.float32)
    emb_d = par("emb", [NEMB, H], dt.float16)
    projW_d = par("projW", [H, H], dt.float16)
    projss_d = par("projss", [128, 256], dt.float32)
    wsrc_d = par("wsrc", [H, L * 256], dt.float16)
    wdhi_d = par("wdhi", [H, L * 256], dt.float16)
    wdlo_d = par("wdlo", [H, L * 256], dt.float16)
    wea_d = par("wea", [ED + 1, L * 256], dt.float16)
    convss_d = par("convss", [128, L * 256], dt.float32)
    gatew1_d = par("gatew1", [H, H // 2], dt.float32)
    gateb14_d = par("gateb14", [128, 256], dt.float32)
    gatew2r4_d = par("gatew2r4", [128, 256], dt.float32)
    gateb2_d = par("gateb2", [128, 1], dt.float32)
    projsc2_d = par("projsc2", [128, 256], dt.float32)
    projsh2_d = par("projsh2", [128, 256], dt.float32)
    headw1_d = par("headw1", [H, H], dt.float32)
    h1ss_d = par("h1ss", [128, 256], dt.float32)
    headw2_d = par("headw2", [H, H // 2], dt.float32)
    h2ss_d = par("h2ss", [128, 128], dt.float32)
    headw3_d = par("headw3", [H // 2, H // 4], dt.float32)
    h3b_d = par("h3b", [128, H // 4], dt.float32)
    headw4_d = par("headw4", [H // 4, 1], dt.float32)
    h4b_d = par("h4b", [128, 1], dt.float32)
    identf_d = par("identf", [128, 128], dt.float32)
    identb_d = par("identb", [128, 128], dt.float16)

    out_d = nc.declare_dram_parameter("out", [GPC, 1], dt.float32, isOutput=True)

    hstage = nc.dram_tensor("hstage", [N_LOC // 2, 256], dt.float16)
    hfull = [
        nc.dram_tensor(f"hfull{i}", [PAIRS, 256], dt.float16,
                       addr_space="Shared")
        for i in range(2)
    ]

    FT, HT = dt.float32, dt.float16
    AF = mybir.ActivationFunctionType
    OP = mybir.AluOpType

    with tile.TileContext(nc) as tc:
        with (
            tc.tile_pool(name="const", bufs=1) as cpool,
            tc.tile_pool(name="state", bufs=1) as spool,
            tc.tile_pool(name="psA", bufs=3, space="PSUM") as psA,   # fs
            tc.tile_pool(name="psB", bufs=1, space="PSUM") as psB,   # p / head
            tc.tile_pool(name="psG", bufs=2, space="PSUM") as psG,   # aggr
        ):
            # ---------------- resident tiles ----------------
            def load(pool, dram, shape, dtp):
                nm = f"c_{dram.name}"
                t = pool.tile(shape, dtp, name=nm, tag=nm)
                nc.sync.dma_start(out=t[:], in_=dram[:])
                return t

            gidx_t = load(cpool, gidx_d, [128, NSLOT // 16], dt.int16)
            wsrc_t = load(cpool, wsrc_d, [H, L * 256], HT)
            wdhi_t = load(cpool, wdhi_d, [H, L * 256], HT)
            wdlo_t = load(cpool, wdlo_d, [H, L * 256], HT)
            wea_t = load(cpool, wea_d, [ED + 1, L * 256], HT)
            convss_t = load(cpool, convss_d, [128, L * 256], FT)
            identf_t = load(cpool, identf_d, [128, 128], FT)
            identb_t = load(cpool, identb_d, [128, 128], HT)

            h_loc = spool.tile([128, NT, H], FT, tag="h_loc")
            h16 = spool.tile([128, NT, H], HT, tag="h16")
            hres = spool.tile([128, NT, H], HT, tag="hres")

            dma_sem = nc.alloc_semaphore("swdge_dma")
            # manual gating for prepared gathers: Tile's deferred-dep path
            # gates neither triggers on the AllGather nor consumers on DMA
            # completion. Consumers wait on dma_sem (+16 per gather DMA,
            # descriptor-embedded). Each layer's triggers are gated on its
            # h table by a small PLAIN gather of hf (Tile makes it wait
            # for the collective; queue order covers the triggers behind).
            n_gather = [0]  # cumulative prepared gathers triggered

            def emit_stage(s, hf_next):
                # cast + stage + AllGather one range slice of the next
                # layer's h table (slices 0-2: 16 ranges, slice 3: 4).
                r0, nr = 16 * s, (16 if s < 3 else 4)
                hl = h_loc[:, r0 : r0 + nr, :].rearrange("p t h -> p (t h)")
                h16s = h16[:, r0 : r0 + nr, :].rearrange("p t h -> p (t h)")
                nc.vector.tensor_scalar_mul(out=h16s, in0=hl, scalar1=HSC)
                nc.vector.scalar_tensor_tensor(
                    out=hres[:, r0 : r0 + nr, :].rearrange("p t h -> p (t h)"),
                    in0=hl, scalar=HSC, in1=h16s,
                    op0=OP.mult, op1=OP.subtract)
                nc.sync.dma_start(
                    out=hstage[1024 * s : 1024 * s + 64 * nr, :]
                    .rearrange("n (two h) -> (n two) h", two=2)
                    .rearrange("(t p) h -> p t h", p=128),
                    in_=h16[:, r0 : r0 + nr, :],
                )
                nc.gpsimd.collective_compute(
                    "AllGather",
                    mybir.AluOpType.bypass,
                    replica_groups=[list(range(C))],
                    ins=[hstage[1024 * s : 1024 * s + 64 * nr, :]],
                    outs=[hf_next[8192 * s : 8192 * s + 512 * nr, :]],
                )

            # ---------------- embedding + projection ----------------
            with (
                tc.tile_pool(name="proj", bufs=2) as prpool,
                tc.tile_pool(name="projc", bufs=1) as prcpool,
                tc.tile_pool(name="psHp", bufs=1, space="PSUM") as psHp,
            ):
                xidx_t = load(prcpool, xidx_d, [128, N_LOC // 16], dt.int16)
                projW_t = load(prcpool, projW_d, [H, H], HT)
                projsc2_t = load(prcpool, projsc2_d, [128, 256], FT)
                projsh2_t = load(prcpool, projsh2_d, [128, 256], FT)
                # gather groups match the 16/16/16/4 staging slices so each
                # slice's cast+AllGather fires as soon as its tiles land
                t0g = 0
                for g, ntt in enumerate((16, 16, 16, 4)):
                    h0h = prpool.tile([128, 16, H], HT, tag="h0h",
                                      name=f"h0h{g}")
                    nc.gpsimd.dma_gather(
                        h0h[:, :ntt, :], emb_d[:],
                        xidx_t[:, t0g * 8 : (t0g + ntt) * 8],
                        ntt * 128, ntt * 128, H, single_packet=False,
                    )
                    for tt2 in range(0, ntt, 2):
                        t0 = t0g + tt2
                        pm = psB.tile([128, 256], FT, tag="pB",
                                      name=f"prm{t0}")
                        for k in range(2):
                            t = t0 + k
                            pT = psHp.tile([128, 128], HT, tag="trh",
                                           name=f"prT{t}")
                            nc.tensor.transpose(pT[:], h0h[:, tt2 + k, :],
                                                identb_t[:])
                            hT = prpool.tile([128, 128], HT, tag="hT16",
                                             name=f"prh{t}")
                            nc.vector.tensor_copy(hT[:], pT[:])
                            nc.tensor.matmul(pm[:, k * H : (k + 1) * H],
                                             hT[:], projW_t[:],
                                             start=True, stop=True)
                        t1 = prpool.tile([128, 256], FT, tag="nupd",
                                         name=f"pru{t0}")
                        nc.vector.tensor_tensor(
                            out=t1[:], in0=pm[:], in1=projsc2_t[:],
                            op=OP.mult)
                        nc.vector.tensor_tensor(
                            out=t1[:], in0=t1[:], in1=projsh2_t[:],
                            op=OP.add)
                        sgp = prpool.tile([128, 256], FT, tag="sgp",
                                          name=f"prs{t0}")
                        nc.scalar.activation(sgp[:], t1[:], AF.Sigmoid)
                        nc.vector.tensor_mul(
                            out=h_loc[:, t0 : t0 + 2, :]
                            .rearrange("p t h -> p (t h)"),
                            in0=t1[:], in1=sgp[:])
                    if _L_RUN > 0:
                        emit_stage(g, hfull[0])
                    t0g += ntt

            # ---------------- conv layers ----------------
            with (
                tc.tile_pool(name="gbuf", bufs=2) as gpool,
                tc.tile_pool(name="work", bufs=2) as wpool,
                tc.tile_pool(name="pq", bufs=1) as pqpool,
                tc.tile_pool(name="mb", bufs=2) as mbpool,
                tc.tile_pool(name="mb1", bufs=1) as mb1pool,
                tc.tile_pool(name="uq", bufs=1) as uqpool,
                tc.tile_pool(name="psHc", bufs=2, space="PSUM") as psHc,
            ):
                for l in range(_L_RUN):
                    hf = hfull[l % 2]
                    lsl = slice(l * 256, (l + 1) * 256)
                    # h16/hres for this layer were cast slice-by-slice
                    # during the previous layer's block loop (or the
                    # projection phase for l=0); the AllGathers are in
                    # flight already.
                    aggr = {}
                    # dst-side projections p = h @ Wdst in ~fp32:
                    # (h16+hres) @ (wdhi+wdlo), hres@wdlo dropped.
                    # Emitted before the collective so the PE chews on them
                    # during the AllGather.
                    p_hi = pqpool.tile([128, NT, 256], HT, tag="p_hi",
                                       name=f"p_hi_{l}")
                    p_lo = pqpool.tile([128, NT, 256], HT, tag="p_lo",
                                       name=f"p_lo_{l}")
                    for t in range(NT):
                        pT = psHc.tile([128, 128], HT, tag="trh",
                                       name=f"pT_{l}_{t}")
                        nc.tensor.transpose(pT[:], h16[:, t, :],
                                            identb_t[:])
                        hTb = wpool.tile([128, 128], HT, tag="hTb",
                                         name=f"hTb_{l}_{t}")
                        nc.scalar.activation(hTb[:], pT[:], AF.Identity)
                        pT2 = psHc.tile([128, 128], HT, tag="trh",
                                        name=f"pT2_{l}_{t}")
                        nc.tensor.transpose(pT2[:], hres[:, t, :],
                                            identb_t[:])
                        hTr = wpool.tile([128, 128], HT, tag="hTr",
                                         name=f"hTr_{l}_{t}")
                        nc.vector.tensor_copy(hTr[:], pT2[:])
                        pm = psB.tile([128, 256], FT, tag="pB",
                                      name=f"pm_{l}_{t}")
                        nc.tensor.matmul(pm[:], hTb[:], wdhi_t[:, lsl],
                                         start=True, stop=False)
                        nc.tensor.matmul(pm[:], hTb[:], wdlo_t[:, lsl],
                                         start=False, stop=False)
                        nc.tensor.matmul(pm[:], hTr[:], wdhi_t[:, lsl],
                                         start=False, stop=True)
                        nc.scalar.activation(p_hi[:, t, :], pm[:],
                                             AF.Identity, scale=1.0 / WSC)
                        if _PLO:
                            nc.vector.scalar_tensor_tensor(
                                out=p_lo[:, t, :], in0=pm[:],
                                scalar=1.0 / WSC, in1=p_hi[:, t, :],
                                op0=OP.mult, op1=OP.subtract)

                    pend = None  # software-pipelined aggregation

                    def emit_aggr(pb, pmsg, pssc, l=l):
                        for j in range(CPB):
                            c = pb * CPB + j
                            r = c // CPR
                            qd = r // 4
                            sl = slice(j * 128, (j + 1) * 128)
                            if c % 16 == 0:
                                aggr[qd] = psG.tile(
                                    [128, 4 * H], FT, tag="aggr",
                                    name=f"aggr_{l}_{qd}")
                            osl = slice((r % 4) * H, (r % 4 + 1) * H)
                            nc.tensor.matmul(
                                aggr[qd][:, osl], pssc[:, sl],
                                pmsg[:, j, :],
                                start=(c % CPR == 0),
                                stop=(c % CPR == CPR - 1))
                            if c % 16 == 15:
                                # quad-batched BN + silu + residual
                                r0 = qd * 4
                                u4 = uqpool.tile(
                                    [128, 4, H], FT, tag="u4",
                                    name=f"u_{l}_{qd}")
                                u4f = u4[:].rearrange("p c h -> p (c h)")
                                h4 = h_loc[:, r0 : r0 + 4, :].rearrange(
                                    "p c h -> p (c h)")
                                nc.vector.tensor_tensor(
                                    out=u4f, in0=aggr[qd][:], in1=h4,
                                    op=OP.add)
                                del aggr[qd]
                                for i in range(4):
                                    nc.vector.tensor_tensor(
                                        out=u4[:, i, :], in0=u4[:, i, :],
                                        in1=convss_t[
                                            :, l * 256 : l * 256 + 128],
                                        op=OP.mult)
                                    nc.vector.tensor_tensor(
                                        out=u4[:, i, :], in0=u4[:, i, :],
                                        in1=convss_t[
                                            :, l * 256 + 128
                                            : (l + 1) * 256],
                                        op=OP.add)
                                us = uqpool.tile(
                                    [128, 4 * H], FT, tag="us",
                                    name=f"us_{l}_{qd}")
                                nc.scalar.activation(us[:], u4f,
                                                     AF.Sigmoid)
                                nc.vector.tensor_mul(out=u4f, in0=u4f,
                                                     in1=us[:])
                                nc.vector.tensor_tensor(
                                    out=h4, in0=h4, in1=u4f, op=OP.add)
                                # next layer's table: stage slice s as
                                # soon as its last qd group (4s+3) lands
                                if l + 1 < _L_RUN and qd in (3, 7, 11):
                                    emit_stage(qd // 4, hfull[(l + 1) % 2])

                    for q in range(4):  # quarters: 13 ranges, 4 blocks each
                        for bq in range(4):
                            b = q * 4 + bq
                            bsl = slice(b * SLOT_B, (b + 1) * SLOT_B)
                            if _PREP and b == 0:
                                # AG fence: a plain 128-slot gather of hf
                                # blocks gpsimd until the table's
                                # AllGathers land (Tile-managed dep).
                                # Must precede the prep: plain descriptors
                                # cannot pass an untriggered ring entry.
                                fence = wpool.tile(
                                    [128, 1, 256], HT, tag="agfence",
                                    name=f"fence_{l}")
                                nc.gpsimd.dma_gather(
                                    fence[:], hf[:], gidx_t[:, :8],
                                    128, 128, 256,
                                    single_packet=False)
                            gb = gpool.tile([128, 2, SLOT_B], HT, tag="gb",
                                            name=f"gb_{l}_{b}")
                            nc.gpsimd.dma_gather(
                                gb[:], hf[:],
                                gidx_t[:, b * (SLOT_B // 16) : (b + 1) * (SLOT_B // 16)],
                                SLOT_B, SLOT_B, 256, transpose=True,
                                single_packet=False,
                                prepare_only=_PREP, sem=dma_sem if _PREP else None,
                            )
                            if _PREP:
                                nc.gpsimd.trigger_dma(count=None)
                                n_gather[0] += 1
                            mask = wpool.tile([128, SLOT_B], dt.uint8,
                                              tag="mask", name=f"mk_{l}_{b}")
                            nc.sync.dma_start(out=mask[:], in_=pmask_d[:, bsl])
                            ea_t = wpool.tile([ED + 1, SLOT_B], HT, tag="ea",
                                              name=f"ea_{l}_{b}")
                            nc.sync.dma_start(out=ea_t[:], in_=eaT_d[:, bsl])
                            ssc_t = wpool.tile([128, SLOT_B], HT, tag="sscb",
                                               name=f"ssc_{l}_{b}")
                            nc.sync.dma_start(out=ssc_t[:], in_=ssc_d[:, bsl])
                            eoh_t = wpool.tile([128, SLOT_B], HT, tag="eohb",
                                               name=f"eoh_{l}_{b}")
                            nc.sync.dma_start(out=eoh_t[:], in_=eoh_d[:, bsl])
                            merged = gpool.tile([128, SLOT_B], HT, tag="mg",
                                                name=f"mg_{l}_{b}")
                            nc.vector.tensor_copy(merged[:], gb[:, 0, :])
                            nc.vector.copy_predicated(merged[:], mask[:],
                                                      gb[:, 1, :])

                            sg_b = mbpool.tile([128, CPB, H], HT, tag="sg_b",
                                               name=f"sg_{l}_{b}")
                            s16_b = mbpool.tile([128, CPB, H], HT, tag="s16",
                                                name=f"s16_{l}_{b}")
                            for j in range(CPB):
                                c = b * CPB + j
                                r = c // CPR
                                sl = slice(j * 128, (j + 1) * 128)
                                fs = psA.tile([128, 256], FT, tag="fs",
                                              name=f"fs_{l}_{c}")
                                nc.tensor.matmul(
                                    fs[:], merged[:, sl], wsrc_t[:, lsl],
                                    start=True, stop=False)
                                nc.tensor.matmul(
                                    fs[:], ea_t[:, sl], wea_t[:, lsl],
                                    start=False, stop=False)
                                nc.tensor.matmul(
                                    fs[:], eoh_t[:, sl], p_hi[:, r, :],
                                    start=False, stop=not _PLO)
                                if _PLO:
                                    nc.tensor.matmul(
                                        fs[:], eoh_t[:, sl], p_lo[:, r, :],
                                        start=False, stop=True)
                                nc.scalar.activation(sg_b[:, j, :], fs[:, :H],
                                                     AF.Sigmoid)
                                nc.vector.tensor_copy(s16_b[:, j, :],
                                                      fs[:, H:])

                            if pend is not None:
                                emit_aggr(*pend)
                                pend = None

                            # block-batched softplus:
                            # sp = max(s, ln(1+exp(min(s,10))))
                            s10 = mb1pool.tile([128, CPB * H], HT, tag="s10",
                                              name=f"s10_{l}_{b}")
                            nc.vector.tensor_scalar_min(
                                out=s10[:],
                                in0=s16_b[:].rearrange("p c h -> p (c h)"),
                                scalar1=10.0)
                            e_b = mb1pool.tile([128, CPB * H], HT, tag="e_b",
                                              name=f"e_{l}_{b}")
                            nc.scalar.activation(e_b[:], s10[:], AF.Exp)
                            sp0 = mb1pool.tile([128, CPB * H], HT, tag="sp0",
                                              name=f"sp0_{l}_{b}")
                            nc.scalar.activation(sp0[:], e_b[:], AF.Ln,
                                                 bias=1.0)
                            spm = mb1pool.tile([128, CPB * H], HT, tag="spm",
                                              name=f"spm_{l}_{b}")
                            nc.vector.tensor_max(
                                out=spm[:], in0=sp0[:],
                                in1=s16_b[:].rearrange("p c h -> p (c h)"))
                            msg = mbpool.tile([128, CPB, H], HT, tag="msg",
                                              name=f"msg_{l}_{b}")
                            nc.vector.tensor_mul(
                                out=msg[:].rearrange("p c h -> p (c h)"),
                                in0=sg_b[:].rearrange("p c h -> p (c h)"),
                                in1=spm[:])

                            pend = (b, msg, ssc_t)

                    emit_aggr(*pend)
                    pend = None
                    if l + 1 < _L_RUN:
                        emit_stage(3, hfull[(l + 1) % 2])

            # ---------------- gate + pooling + head ----------------
            with (
                tc.tile_pool(name="poolc", bufs=1) as pcpool,
                tc.tile_pool(name="pools", bufs=3) as smpool,
                tc.tile_pool(name="psP", bufs=2, space="PSUM") as psP,
            ):
                goh_t = load(pcpool, goh_d, [128, NT * GPC], FT)
                goh2_t = load(pcpool, goh2_d, [GPC, N_LOC], FT)
                maskb_t = load(pcpool, maskbias_d, [128, NT * GPC], FT)
                gatew1_t = load(pcpool, gatew1_d, [H, H // 2], FT)
                gateb14_t = load(pcpool, gateb14_d, [128, 256], FT)
                gatew2r4_t = load(pcpool, gatew2r4_d, [128, 256], FT)
                gateb2_t = load(pcpool, gateb2_d, [128, 1], FT)
                headw1_t = load(pcpool, headw1_d, [H, H], FT)
                h1ss_t = load(pcpool, h1ss_d, [128, 256], FT)
                headw2_t = load(pcpool, headw2_d, [H, H // 2], FT)
                h2ss_t = load(pcpool, h2ss_d, [128, 128], FT)
                headw3_t = load(pcpool, headw3_d, [H // 2, H // 4], FT)
                h3b_t = load(pcpool, h3b_d, [128, H // 4], FT)
                headw4_t = load(pcpool, headw4_d, [H // 4, 1], FT)
                h4b_t = load(pcpool, h4b_d, [128, 1], FT)

                g_all = pcpool.tile([128, NT], FT, name="g_all", tag="g_all")
                runmax = pcpool.tile([128, GPC], FT, name="runmax",
                                     tag="runmax")

                # pass 1: gate scores g, 4 tiles per round; the per-graph
                # max falls out of one masked [128, NT*GPC] array at the end
                gmx = pcpool.tile([128, NT * GPC], FT, name="gmx", tag="gmx")
                for gq in range(NT // 4):
                    g1 = psB.tile([128, 256], FT, tag="pB", name=f"g1_{gq}")
                    for i in range(4):
                        t = 4 * gq + i
                        pT = psP.tile([128, 128], FT, tag="tr", name=f"gT{t}")
                        nc.tensor.transpose(pT[:], h_loc[:, t, :],
                                            identf_t[:])
                        hT = smpool.tile([128, 128], FT, tag="hT32",
                                         name=f"gh{t}")
                        nc.vector.tensor_copy(hT[:], pT[:])
                        nc.tensor.matmul(g1[:, i * 64 : i * 64 + 64], hT[:],
                                         gatew1_t[:], start=True, stop=True)
                    s1 = smpool.tile([128, 256], FT, tag="s1",
                                     name=f"s1_{gq}")
                    nc.vector.tensor_tensor(out=s1[:], in0=g1[:],
                                            in1=gateb14_t[:], op=OP.add)
                    s1s = smpool.tile([128, 256], FT, tag="s1s",
                                      name=f"s1s_{gq}")
                    nc.scalar.activation(s1s[:], s1[:], AF.Silu)
                    nc.vector.tensor_mul(out=s1s[:], in0=s1s[:],
                                         in1=gatew2r4_t[:])
                    nc.vector.tensor_reduce(
                        out=g_all[:, 4 * gq : 4 * gq + 4]
                        .rearrange("p (t o) -> p t o", o=1),
                        in_=s1s[:].rearrange("p (t k) -> p t k", t=4),
                        axis=mybir.AxisListType.X, op=OP.add)
                    nc.vector.tensor_scalar(
                        out=g_all[:, 4 * gq : 4 * gq + 4],
                        in0=g_all[:, 4 * gq : 4 * gq + 4],
                        scalar1=gateb2_t[:], scalar2=None, op0=OP.add)
                nc.vector.tensor_tensor(
                    out=gmx[:].rearrange("p (t k) -> p t k", k=GPC),
                    in0=g_all[:].rearrange("p (t o) -> p t o", o=1)
                    .to_broadcast([128, NT, GPC]),
                    in1=goh_t[:].rearrange("p (t k) -> p t k", k=GPC),
                    op=OP.mult)
                nc.vector.tensor_tensor(out=gmx[:], in0=gmx[:],
                                        in1=maskb_t[:], op=OP.add)
                nc.vector.tensor_reduce(
                    out=runmax[:],
                    in_=gmx[:].rearrange("p (t k) -> p k t", k=GPC),
                    axis=mybir.AxisListType.X, op=OP.max)

                # reduce running max across partitions -> gmax [GPC, 1]
                pTm = psP.tile([128, 128], FT, tag="tr", name="pTm")
                nc.tensor.transpose(pTm[:GPC, :], runmax[:], identf_t[:])
                rmT = smpool.tile([GPC, 128], FT, tag="rmT", name="rmT")
                nc.vector.tensor_copy(rmT[:], pTm[:GPC, :])
                negmax = smpool.tile([GPC, 1], FT, tag="negmax",
                                     name="negmax")
                nc.vector.tensor_reduce(out=negmax[:], in_=rmT[:],
                                        axis=mybir.AxisListType.X,
                                        op=OP.max)
                nc.vector.tensor_scalar_mul(out=negmax[:], in0=negmax[:],
                                            scalar1=-1.0)

                # pass 2: e = exp(min(g - gmax[graph], 20)) batched across
                # all tiles, then the pooled-sum matmuls
                eps_ps = psB.tile([128, 256], FT, tag="pB", name="eps")
                for t in range(NT):
                    nc.tensor.matmul(
                        eps_ps[:, t : t + 1],
                        goh2_t[:, t * 128 : (t + 1) * 128],
                        negmax[:], start=True, stop=True)
                earg = smpool.tile([128, NT], FT, tag="earg", name="earg")
                nc.vector.tensor_tensor(out=earg[:], in0=g_all[:],
                                        in1=eps_ps[:, :NT], op=OP.add)
                nc.vector.tensor_scalar_min(out=earg[:], in0=earg[:],
                                            scalar1=20.0)
                e_all = smpool.tile([128, NT], FT, tag="ecol", name="e_all")
                nc.scalar.activation(e_all[:], earg[:], AF.Exp)
                pool_ps = psA.tile([GPC, H + 1], FT, tag="fs", name="pool_ps")
                for t in range(NT):
                    rhs = smpool.tile([128, H + 1], FT, tag="rhs",
                                      name=f"rhs_{t}")
                    nc.vector.tensor_scalar(
                        out=rhs[:, :H], in0=h_loc[:, t, :],
                        scalar1=e_all[:, t : t + 1],
                        scalar2=None, op0=OP.mult)
                    nc.vector.tensor_copy(rhs[:, H : H + 1],
                                          e_all[:, t : t + 1])
                    nc.tensor.matmul(
                        pool_ps[:], goh_t[:, t * GPC : (t + 1) * GPC], rhs[:],
                        start=(t == 0), stop=(t == NT - 1))

                pooled_raw = smpool.tile([GPC, H + 1], FT, tag="praw")
                nc.vector.tensor_copy(pooled_raw[:], pool_ps[:])
                rec = smpool.tile([GPC, 1], FT, tag="rec")
                nc.vector.reciprocal(rec[:], pooled_raw[:, H : H + 1])
                pooled = smpool.tile([GPC, H], FT, tag="pooled")
                nc.vector.tensor_scalar(
                    out=pooled[:], in0=pooled_raw[:, :H], scalar1=rec[:],
                    scalar2=None, op0=OP.mult)

                def head_mm(x, w, nin, nout, nm, ss=None, badd=None,
                            silu=True):
                    pT = psP.tile([128, 128], FT, tag="tr",
                                  name=f"hT{nm}")
                    nc.tensor.transpose(pT[:nin, :GPC], x[:],
                                        identf_t[:GPC, :GPC])
                    xT = smpool.tile([128, GPC], FT, tag="xT",
                                     name=f"xT{nm}")
                    nc.vector.tensor_copy(xT[:nin, :], pT[:nin, :GPC])
                    ym = psB.tile([128, 256], FT, tag="pB", name=f"ym{nm}")
                    nc.tensor.matmul(ym[:GPC, :nout], xT[:nin, :], w[:],
                                     start=True, stop=True)
                    y = smpool.tile([GPC, nout], FT, tag=f"hd{nout}",
                                    name=f"y{nm}")
                    if ss is not None:
                        nc.vector.tensor_tensor(
                            out=y[:], in0=ym[:GPC, :nout],
                            in1=ss[:GPC, :nout], op=OP.mult)
                        nc.vector.tensor_tensor(
                            out=y[:], in0=y[:], in1=ss[:GPC, nout : 2 * nout],
                            op=OP.add)
                    elif badd is not None:
                        nc.vector.tensor_tensor(
                            out=y[:], in0=ym[:GPC, :nout],
                            in1=badd[:GPC, :nout], op=OP.add)
                    else:
                        nc.vector.tensor_copy(y[:], ym[:GPC, :nout])
                    if silu:
                        ysig = smpool.tile([GPC, nout], FT,
                                           tag=f"hs{nout}", name=f"ys{nm}")
                        nc.scalar.activation(ysig[:], y[:], AF.Sigmoid)
                        nc.vector.tensor_mul(out=y[:], in0=y[:], in1=ysig[:])
                    return y

                y1 = head_mm(pooled, headw1_t, H, H, "a", ss=h1ss_t)
                y2 = head_mm(y1, headw2_t, H, H // 2, "b", ss=h2ss_t)
                y3 = head_mm(y2, headw3_t, H // 2, H // 4, "c", badd=h3b_t)
                y4 = head_mm(y3, headw4_t, H // 4, 1, "d", badd=h4b_t,
                             silu=False)
                nc.sync.dma_start(out=out_d[:], in_=y4[:])

    return nc


_NC_CACHE = None
_LAST_EXEC_NS = None


def kernel(**inputs) -> np.ndarray:
    global _NC_CACHE, _LAST_EXEC_NS
    in_maps = _prep(inputs)
    if _NC_CACHE is None:
        _NC_CACHE = _build()
        _NC_CACHE.finalize()
    trace = os.environ.get("KERNEL_TRACE", "0") == "1"
    res = run_bass_kernel_spmd(
        _NC_CACHE, in_maps, core_ids=list(range(C)), trace=trace,
        tmpdir=os.environ.get("KERNEL_TRACE_DIR") if trace else None,
    )
    _LAST_EXEC_NS = res.exec_time_ns
    out = np.concatenate(
        [np.asarray(res.results[c]["out"]).reshape(GPC) for c in range(C)]
    )
    return out.astype(F32)


if __name__ == "__main__":
    import jax

    with jax.default_device(jax.devices("cpu")[0]):
        sys.path.insert(0, os.path.dirname(os.path.abspath(__file__)))
        import reference

        inp = {k: np.asarray(v) for k, v in reference.setup_inputs().items()}
    y = kernel(**inp)
    print("out[:8]:", y[:8])

